# revision 14
# baseline (speedup 1.0000x reference)
# Trainium2 Bass kernel for nn_CustomAttention (fused qkv + LoRA + per-head
# LayerNorm + softmax attention + output projection).
#
# Sharding: 16 heads split across 8 cores (2 heads/core), both batch elements
# on every core. Each core computes its heads' attention and its partial
# output projection (sum over its heads' columns); the host sums the 8
# bf16 partials in f32 and adds proj_b. LoRA is folded into the qkv weights
# on the host:  x@W.T + (x@A)@B*s == x@(W + s*(A@B).T).T
#
# v2 design notes (cost model: PE sequencer ~0.43ns/streamed column +
# ~13ns/matmul; Act exp ~0.87ns/col + 445ns/inst; DVE ~0.25-1.2ns/col +
# ~250ns/inst; Pool(gpsimd) ~1.6ns/col, mostly idle -> gets bulk copies):
#  - scores are computed transposed (sT[j,i]) so softmax-normalized output
#    comes out in [d, i] layout = exactly the lhsT the projection needs.
#  - exp without max subtraction (softmax shift-invariance; post-LN scores
#    are bounded so fp32 exp cannot overflow).
#  - attention@v stationary is [v_h0 | ones | v_h1] (192 cols): h0 uses
#    cols 0:128 = [v|1], h1 uses cols 64:192 = [1|v]. So av_h0 has out^T in
#    partitions 0:64 / denom in 64:128, and av_h1 the mirror image. The
#    normalized oT2 tile then holds h0 rows 0:64 and h1 rows 64:128 with
#    all reads same-partition -> the projection contracts K=128 (both
#    heads) in a single pass.
#  - q/k transposes are 2x [128,128] PE transposes per row-tile (q and k
#    each packed across both heads), not 4x [128,64].
#  - phase B runs i-blocks of 1024 with F=512 matmuls and [128,1024] exp
#    tiles (amortizes the 445ns/inst Act overhead).
#  - engine placement: QKV-psum drain + v-copies + sqs on Pool, LN stats +
#    nat + oT/recip + osb on DVE, qkT psum drain on Act (idle in phase A),
#    exp on Act.
import numpy as np
import ml_dtypes

import concourse.bass as bass
import concourse.bacc as bacc
import concourse.mybir as mybir
from concourse.tile import TileContext
from concourse.masks import make_identity
from concourse.bass_utils import run_bass_kernel_spmd

BF16 = ml_dtypes.bfloat16
F32 = np.float32

B, N, DIM, H, R = 2, 2048, 1024, 16, 8
D = DIM // H              # 64
NCORES = 8
HPC = H // NCORES         # 2 heads per core
ALPHA = 8.0
LORA_SCALE = ALPHA / R
EPS = 1e-5
QSCALE = float(D) ** -0.5  # 0.125

NCH = DIM // 128          # 8 contraction chunks of 128
NTI = N // 128            # 16 row tiles of 128
HT = NTI // 2             # row tiles per stats half
IBW = 1024                # i-block width in phase B
NIB = N // IBW            # 2 i-blocks

_prog_cache: dict = {}


def _build_program(use_mask: bool, affine_q: bool, affine_k: bool, repeat: int = 1,
                   phases: str = "AB"):
    nc = bacc.Bacc("TRN2", target_bir_lowering=False)
    f32 = mybir.dt.float32
    bf16 = mybir.dt.bfloat16

    xT = nc.dram_tensor("xT", [128, B, NCH, N], bf16, kind="ExternalInput")
    wT = nc.dram_tensor("wT", [NCH, 128, 6 * D], bf16, kind="ExternalInput")
    projT = nc.dram_tensor("projT", [128, DIM], bf16, kind="ExternalInput")
    out_p = nc.dram_tensor("out_p", [128, B, NTI, DIM], bf16, kind="ExternalOutput")
    if affine_q or affine_k:
        # rows: 0=qw*scale 1=qb*scale 2=kw 3=kb, each broadcast to 128 parts
        lnaff = nc.dram_tensor("lnaff", [4, 128, D], f32, kind="ExternalInput")
    if use_mask:
        emaskT = nc.dram_tensor("emaskT", [N, N], bf16, kind="ExternalInput")

    with TileContext(nc) as tc:
        import contextlib
        with contextlib.ExitStack() as ctx:
            const = ctx.enter_context(tc.tile_pool(name="const", bufs=1))
            ident = const.tile([128, 128], bf16)
            make_identity(nc, ident)
            eps_t = const.tile([128, 1], f32)
            nc.vector.memset(eps_t, EPS)

            persist = ctx.enter_context(tc.tile_pool(name="persist", bufs=1))
            w_sb = persist.tile([128, NCH, 6 * D], bf16)
            nc.sync.dma_start(out=w_sb, in_=wT.rearrange("ci cm w -> cm ci w"))
            proj_sb = persist.tile([128, DIM], bf16)
            nc.sync.dma_start(out=proj_sb, in_=projT[:, :])
            if affine_q or affine_k:
                aff_sb = persist.tile([128, 4, D], f32)
                nc.sync.dma_start(out=aff_sb, in_=lnaff.rearrange("r p d -> p r d"))

            xpool = ctx.enter_context(tc.tile_pool(name="xpool", bufs=2))
            qkpool = ctx.enter_context(tc.tile_pool(name="qkpool", bufs=2))
            vpool = ctx.enter_context(tc.tile_pool(name="vpool", bufs=2))
            stgp = ctx.enter_context(tc.tile_pool(name="stgp", bufs=2))
            sqsp = ctx.enter_context(tc.tile_pool(name="sqsp", bufs=2))
            lnp = ctx.enter_context(tc.tile_pool(name="lnp", bufs=2))
            natp = ctx.enter_context(tc.tile_pool(name="natp", bufs=4))
            esp = ctx.enter_context(tc.tile_pool(name="esp", bufs=3))
            oTp = ctx.enter_context(tc.tile_pool(name="oTp", bufs=2))
            zrp = ctx.enter_context(tc.tile_pool(name="zrp", bufs=2))
            osbp = ctx.enter_context(tc.tile_pool(name="osbp", bufs=3))
            if use_mask:
                mskp = ctx.enter_context(tc.tile_pool(name="mskp", bufs=2))

            # sT ping/pong 4 banks; av 2 banks; pp 2 banks. Phase A's pq/pt
            # borrow the sT rotation (A and B never overlap in-flight).
            psS = ctx.enter_context(tc.tile_pool(name="psS", bufs=2, space="PSUM"))
            psAV = ctx.enter_context(tc.tile_pool(name="psAV", bufs=1, space="PSUM"))
            psP = ctx.enter_context(tc.tile_pool(name="psP", bufs=1, space="PSUM"))

            if repeat > 1:
                ctx.enter_context(tc.For_i(
                    0, repeat, 1,
                    hint_engines=(mybir.EngineType.PE, mybir.EngineType.SP,
                                  mybir.EngineType.Activation,
                                  mybir.EngineType.DVE, mybir.EngineType.Pool)))

            # hoist both batches' input loads so b=1 prefetches under b=0
            x_sbs = []
            for b in range(B):
                x_sb = xpool.tile([128, NCH, N], bf16, tag="x_sb")
                for nq in range(4):
                    nc.sync.dma_start(out=x_sb[:, :, nq * 512:(nq + 1) * 512],
                                      in_=xT[:, b, :, nq * 512:(nq + 1) * 512])
                x_sbs.append(x_sb)

            qkTs, vp3s, stages = [], [], []
            # ---------------- phase A: qkv gen + LN + transposes ------------
            # First both batches' qkv matmuls back-to-back (PE stays busy
            # while b0's LN-stats chain drains on Pool/DVE/Act), then the
            # stats + normalize + transpose sections.
            for b in range(B):
                x_sb = x_sbs[b]
                # qkT: [p = h0 d | h1 d, 2 = (q, k), n]
                qkT = qkpool.tile([128, 2, N], bf16, tag="qkT")
                # vp3: [j_mod, chunk, 3, 64] = [v_h0 | ones | v_h1]
                vp3 = vpool.tile([128, NTI, 3, D], bf16, tag="vp3")
                nc.vector.memset(vp3[:, :, 1, :], 1.0)
                qkTs.append(qkT)
                vp3s.append(vp3)
                if "A" not in phases:  # timing variant: fill A outputs
                    nc.vector.memset(qkT, 0.5)
                    nc.vector.memset(vp3[:, :, 0:3:2, :], 0.5)
                    stages.append(None)
                    continue

                stage = stgp.tile([128, NTI, 6 * D], f32, tag="stage")
                stages.append(stage)
                for ti in range(NTI):
                    pq = psS.tile([128, 6 * D], f32, tag="sT")
                    for ci in range(NCH):
                        nc.tensor.matmul(
                            pq,
                            lhsT=x_sb[:, ci, ti * 128:(ti + 1) * 128],
                            rhs=w_sb[:, ci, :],
                            start=(ci == 0),
                            stop=(ci == NCH - 1),
                        )
                    # psum -> sbuf drain (GPSIMD cannot read PSUM on hw)
                    if ti % 2 == 0:
                        nc.vector.tensor_copy(out=stage[:, ti, :], in_=pq)
                    else:
                        nc.scalar.copy(out=stage[:, ti, :], in_=pq)
                    # v for both heads: stage cols [128:192] and [320:384]
                    nc.gpsimd.tensor_copy(
                        out=vp3[:, ti, 0:3:2, :],
                        in_=stage.rearrange("p t (h x) -> p t h x", h=2)
                        [:, ti, :, 2 * D:3 * D])

            for b in range(B):
                if "A" not in phases:
                    continue
                qkT, vp3, stage = qkTs[b], vp3s[b], stages[b]
                st5 = stage.rearrange("p t (h i x) -> p t h i x", h=2, i=3)
                for half in range(2):
                    hsl = slice(half * HT, (half + 1) * HT)
                    qkv_v = st5[:, hsl, :, 0:2, :]       # [128, HT, 2, 2, 64]
                    mean = lnp.tile([128, HT, 2, 2], f32, tag="mean")
                    nc.vector.tensor_reduce(
                        out=mean, in_=qkv_v, axis=mybir.AxisListType.X,
                        op=mybir.AluOpType.add)
                    nc.vector.tensor_scalar(
                        out=mean, in0=mean, scalar1=1.0 / D, scalar2=None,
                        op0=mybir.AluOpType.mult)
                    sqs = sqsp.tile([128, HT, 2, 2, D], f32, tag="sqs")
                    nc.vector.tensor_tensor(
                        out=sqs, in0=qkv_v, in1=qkv_v, op=mybir.AluOpType.mult)
                    var = lnp.tile([128, HT, 2, 2], f32, tag="var")
                    nc.vector.tensor_reduce(
                        out=var, in_=sqs, axis=mybir.AxisListType.X,
                        op=mybir.AluOpType.add)
                    m2 = lnp.tile([128, HT, 2, 2], f32, tag="m2")
                    nc.vector.tensor_tensor(
                        out=m2, in0=mean, in1=mean, op=mybir.AluOpType.mult)
                    nc.vector.tensor_scalar(
                        out=var, in0=var, scalar1=1.0 / D, scalar2=None,
                        op0=mybir.AluOpType.mult)
                    nc.vector.tensor_tensor(
                        out=var, in0=var, in1=m2, op=mybir.AluOpType.subtract)
                    rstd = lnp.tile([128, HT, 2, 2], f32, tag="rstd")
                    nc.scalar.activation(
                        out=rstd, in_=var,
                        func=mybir.ActivationFunctionType.Sqrt,
                        bias=eps_t, scale=1.0)
                    nc.vector.reciprocal(out=rstd, in_=rstd)
                    if not affine_q:  # fold q scaling (D^-0.5) into rstd
                        nc.vector.tensor_scalar(
                            out=rstd[:, :, :, 0:1], in0=rstd[:, :, :, 0:1],
                            scalar1=QSCALE, scalar2=None,
                            op0=mybir.AluOpType.mult)
                    # mrs = -mean*rstd, the bias form for Act-engine nats
                    mrs = lnp.tile([128, HT, 2, 2], f32, tag="mrs")
                    nc.vector.tensor_tensor(
                        out=mrs, in0=mean, in1=rstd, op=mybir.AluOpType.mult)
                    nc.vector.tensor_scalar(
                        out=mrs, in0=mrs, scalar1=-1.0, scalar2=None,
                        op0=mybir.AluOpType.mult)

                    for tih in range(HT):
                        ti = half * HT + tih
                        nat_q = natp.tile([128, 2, D], bf16, tag="natq")
                        nat_k = natp.tile([128, 2, D], bf16, tag="natk")
                        for hh in range(2):
                            for qk in range(2):
                                dst = (nat_q if qk == 0 else nat_k)[:, hh, :]
                                src = st5[:, ti, hh, qk, :]
                                affine = affine_q if qk == 0 else affine_k
                                if affine:
                                    natf = natp.tile([128, D], f32, tag="natf")
                                    nc.vector.tensor_scalar(
                                        out=natf, in0=src,
                                        scalar1=mean[:, tih, hh, qk:qk + 1],
                                        scalar2=rstd[:, tih, hh, qk:qk + 1],
                                        op0=mybir.AluOpType.subtract,
                                        op1=mybir.AluOpType.mult)
                                    r = 0 if qk == 0 else 2
                                    natf2 = natp.tile([128, D], f32, tag="natf2")
                                    nc.vector.tensor_tensor(
                                        out=natf2, in0=natf, in1=aff_sb[:, r, :],
                                        op=mybir.AluOpType.mult)
                                    nc.vector.tensor_tensor(
                                        out=dst, in0=natf2, in1=aff_sb[:, r + 1, :],
                                        op=mybir.AluOpType.add)
                                elif qk == 1 and hh == 0 or (qk == 0 and hh == 1):
                                    # spread the nat ops across 3 engines
                                    nc.gpsimd.tensor_scalar(
                                        out=dst, in0=src,
                                        scalar1=mean[:, tih, hh, qk:qk + 1],
                                        scalar2=rstd[:, tih, hh, qk:qk + 1],
                                        op0=mybir.AluOpType.subtract,
                                        op1=mybir.AluOpType.mult)
                                elif qk == 1 and hh == 1:
                                    nc.scalar.activation(
                                        out=dst, in_=src,
                                        func=mybir.ActivationFunctionType.Identity,
                                        scale=rstd[:, tih, hh, qk:qk + 1],
                                        bias=mrs[:, tih, hh, qk:qk + 1])
                                else:
                                    nc.vector.tensor_scalar(
                                        out=dst, in0=src,
                                        scalar1=mean[:, tih, hh, qk:qk + 1],
                                        scalar2=rstd[:, tih, hh, qk:qk + 1],
                                        op0=mybir.AluOpType.subtract,
                                        op1=mybir.AluOpType.mult)
                        pt = psS.tile([128, 2, 128], bf16, tag="sT")
                        nc.tensor.transpose(
                            pt[:, 0, :], nat_q.rearrange("p h x -> p (h x)"), ident)
                        nc.tensor.transpose(
                            pt[:, 1, :], nat_k.rearrange("p h x -> p (h x)"), ident)
                        # Act is idle during phase A; let it drain the psT psum
                        nc.scalar.copy(
                            out=qkT[:, :, ti * 128:(ti + 1) * 128], in_=pt)

            # ---------------- phase B: attention + projection ---------------
            if "B" not in phases:
                nc.compile()
                return nc

            proj_q: list = []  # pending (b, oT2, global_ti, sub)

            def emit_proj_sub():
                pb, poT2, pti, psub = proj_q.pop(0)
                pp = psP.tile([128, DIM], f32, tag="pp")
                for nh in range(2):
                    nc.tensor.matmul(
                        pp[:, nh * 512:(nh + 1) * 512],
                        lhsT=poT2[:, psub * 128:(psub + 1) * 128],
                        rhs=proj_sb[:, nh * 512:(nh + 1) * 512],
                        start=True, stop=True,
                    )
                osb = osbp.tile([128, DIM], bf16, tag="osb")
                nc.vector.tensor_copy(out=osb, in_=pp)
                nc.sync.dma_start(out=out_p[:, pb, pti, :], in_=osb)

            for b in range(B):
                qkT, vp3 = qkTs[b], vp3s[b]
                for iblk in range(NIB):
                    i0 = iblk * IBW
                    oT2 = oTp.tile([128, IBW], bf16, tag="oT2")
                    for hh in range(HPC):
                        hs = slice(hh * D, (hh + 1) * D)
                        av = psAV.tile([128, IBW], f32, tag="av")
                        for j in range(NTI):
                            sT = psS.tile([128, IBW], f32, tag="sT")
                            for e in range(IBW // 512):
                                nc.tensor.matmul(
                                    sT[:, e * 512:(e + 1) * 512],
                                    lhsT=qkT[hs, 1, j * 128:(j + 1) * 128],
                                    rhs=qkT[hs, 0, i0 + e * 512:i0 + (e + 1) * 512],
                                    start=True, stop=True,
                                )
                            es = esp.tile([128, IBW], bf16, tag="es")
                            nc.scalar.activation(
                                out=es, in_=sT,
                                func=mybir.ActivationFunctionType.Exp,
                            )
                            if use_mask:
                                msk = mskp.tile([128, IBW], bf16, tag="msk")
                                nc.sync.dma_start(
                                    out=msk,
                                    in_=emaskT[j * 128:(j + 1) * 128, i0:i0 + IBW],
                                )
                                nc.vector.tensor_tensor(
                                    out=es, in0=es, in1=msk,
                                    op=mybir.AluOpType.mult,
                                )
                            for e in range(IBW // 512):
                                nc.tensor.matmul(
                                    av[:, e * 512:(e + 1) * 512],
                                    lhsT=vp3[:, j, hh:hh + 2, :],
                                    rhs=es[:, e * 512:(e + 1) * 512],
                                    start=(j == 0), stop=(j == NTI - 1),
                                )
                            # drip pending projection work into the PE slack
                            # of the Act-paced attention loop
                            if j % 2 == 1 and proj_q:
                                emit_proj_sub()
                        # normalize: h0 out^T in av[0:64], denom in av[64:128];
                        # h1 mirrored.
                        zr = zrp.tile([D, IBW], f32, tag="zr")
                        nc.vector.reciprocal(
                            out=zr, in_=av[D:, :] if hh == 0 else av[0:D, :])
                        nc.vector.tensor_tensor(
                            out=oT2[hs, :],
                            in0=av[0:D, :] if hh == 0 else av[D:, :],
                            in1=zr, op=mybir.AluOpType.mult)
                    ti0 = iblk * (IBW // 128)
                    for sub in range(IBW // 128):
                        proj_q.append((b, oT2, ti0 + sub, sub))
            while proj_q:
                emit_proj_sub()
    nc.compile()
    return nc


def _prep_inputs(inputs):
    x = np.ascontiguousarray(inputs["x"], dtype=F32)
    qkv_w = np.asarray(inputs["qkv_w"], dtype=F32)
    proj_w = np.asarray(inputs["proj_w"], dtype=F32)
    W_eff = qkv_w.copy()
    for i, (a, bm) in enumerate([("lora_Aq", "lora_Bq"), ("lora_Ak", "lora_Bk"),
                                 ("lora_Av", "lora_Bv")]):
        A = np.asarray(inputs[a], dtype=F32)
        Bm = np.asarray(inputs[bm], dtype=F32)
        W_eff[i * DIM:(i + 1) * DIM] += LORA_SCALE * (A @ Bm).T

    # [cm, b, ci, n] with cm = c % 128, ci = c // 128
    xT_all = np.ascontiguousarray(
        x.transpose(2, 0, 1).reshape(NCH, 128, B, N)
        .transpose(1, 2, 0, 3).astype(BF16))

    qn_w = np.asarray(inputs["qn_w"], F32); qn_b = np.asarray(inputs["qn_b"], F32)
    kn_w = np.asarray(inputs["kn_w"], F32); kn_b = np.asarray(inputs["kn_b"], F32)
    affine_q = not (np.all(qn_w == 1.0) and np.all(qn_b == 0.0))
    affine_k = not (np.all(kn_w == 1.0) and np.all(kn_b == 0.0))
    mask = np.asarray(inputs["attn_mask"], F32)
    use_mask = bool(np.any(mask))

    common = {"xT": xT_all}
    if affine_q or affine_k:
        aff = np.stack([
            np.broadcast_to(qn_w * QSCALE, (128, D)),
            np.broadcast_to(qn_b * QSCALE, (128, D)),
            np.broadcast_to(kn_w, (128, D)),
            np.broadcast_to(kn_b, (128, D)),
        ]).astype(F32)
        common["lnaff"] = np.ascontiguousarray(aff)
    if use_mask:
        common["emaskT"] = np.ascontiguousarray(
            np.exp(mask[0, 0].T).astype(BF16))

    in_maps = []
    for c in range(NCORES):
        h0 = c * HPC
        blocks = []
        for hh in range(HPC):
            h = h0 + hh
            for part in range(3):  # q, k, v
                blocks.append(W_eff[part * DIM + h * D: part * DIM + (h + 1) * D])
        Wlocal = np.concatenate(blocks, axis=0)          # [384, 1024]
        wT_c = np.ascontiguousarray(
            Wlocal.T.reshape(NCH, 128, 6 * D).astype(BF16))
        projT_c = np.ascontiguousarray(np.concatenate(
            [proj_w[:, (h0 + hh) * D:(h0 + hh + 1) * D].T for hh in range(HPC)],
            axis=0).astype(BF16))                        # [128, 1024]
        m = dict(common)
        m["wT"] = wT_c
        m["projT"] = projT_c
        in_maps.append(m)
    return in_maps, (use_mask, affine_q, affine_k)


def _run(inputs, trace=False):
    in_maps, key = _prep_inputs(inputs)
    if key not in _prog_cache:
        _prog_cache[key] = _build_program(*key)
    nc = _prog_cache[key]
    res = run_bass_kernel_spmd(nc, in_maps, core_ids=list(range(NCORES)),
                               trace=trace)
    acc = np.zeros((128, B, NTI, DIM), dtype=F32)
    for r in res.results:
        acc += r["out_p"].astype(F32)
    # [cm, b, ti, c] -> [b, ti*128+cm, c]
    out = np.ascontiguousarray(acc.transpose(1, 2, 0, 3).reshape(B, N, DIM))
    out += np.asarray(inputs["proj_b"], F32)
    return out, res


def kernel(**inputs) -> np.ndarray:
    out, _ = _run(inputs)
    return out
